# revision 12
# baseline (speedup 1.0000x reference)
"""Trainium2 Bass kernel for multi-head causal attention.

Problem: B=2, H=16, S=2048, D=64, fp32, additive causal mask.
Sharding: B*H = 32 heads -> 4 heads per core across 8 cores (no cross-core
communication).

Per-core algorithm (heads processed in row-tile-packed pairs):
  - Q^T, K^T [64, S] built on-chip via PE transposes (fp32 -> fp32r rounded
    by the DVE PSUM->SBUF copy).
  - Scores are computed TRANSPOSED: S^T[k, q] = (K^T)^T-chunk.T @ Q^T via
    fp32r matmuls, two heads packed in the 128-row PE array (contraction
    dim is d=64 per head).  Causally trimmed: for k-tile kt only
    q >= 128*kt is computed.
  - exp via ACT (scale=1/8 folded in, no max-subtraction needed: scores are
    O(6)), writing P^T tiles in bf16.  Diagonal 128x128 blocks get a
    multiplicative upper-triangular 0/1 mask.
  - PV: out^T[d, q] accumulated in PSUM over k-chunks with V (as stored,
    bf16, augmented with a ones column -> row 64 is softmax denominator).
  - normalize: reciprocal of row 64, replicated across partitions with a
    K=1 ones matmul, multiplied on DVE; host transposes [d, q] -> [q, d].
"""

import numpy as np

import concourse.bass as bass
import concourse.mybir as mybir
import concourse.tile as tile
from concourse import bacc
from concourse.bass_utils import run_bass_kernel_spmd
from concourse.masks import make_identity, make_upper_triangular

B = 2
H = 16
S = 2048
D = 64
EMBED = H * D
N_CORES = 8
HPC = (B * H) // N_CORES  # heads per core = 4
NT = S // 128  # 16 k/q tiles of 128
SCALE = float(D) ** -0.5  # 0.125
NEG = -1e9

F32 = mybir.dt.float32
F32R = mybir.dt.float32r
BF16 = mybir.dt.bfloat16


def _ptoff(kt: int) -> int:
    """Column offset of k-tile kt's row-chunk inside a per-head P^T tile.

    Chunk kt covers global q in [128*kt, S) and is stored at local offset
    q - 128*kt."""
    return kt * S - 128 * (kt * (kt - 1) // 2)


PT_W = _ptoff(NT)  # 17408 columns total (causal)


def _build(causal: bool = True) -> bacc.Bacc:
    nc = bacc.Bacc("TRN2", target_bir_lowering=False, debug=False,
                   num_devices=N_CORES)

    q_d = nc.declare_dram_parameter("q", [HPC, S, D], F32, isOutput=False)
    k_d = nc.declare_dram_parameter("k", [HPC, S, D], F32, isOutput=False)
    v_d = nc.declare_dram_parameter("v", [HPC, S, D], F32, isOutput=False)
    if not causal:
        # mask^T for this core's batch: maskT[k, q] = mask[b, 0, q, k]
        mt_d = nc.declare_dram_parameter("maskT", [S, S], F32, isOutput=False)
    out_d = nc.declare_dram_parameter("outT", [HPC, D, S], F32, isOutput=True)

    with tile.TileContext(nc) as tc:
        with (
            tc.tile_pool(name="const", bufs=1) as const_pool,
            tc.tile_pool(name="stage", bufs=12) as stage_pool,
            tc.tile_pool(name="qt", bufs=2 if causal else 1) as qt_pool,
            tc.tile_pool(name="ktp", bufs=2 if causal else 1) as kt_pool,
            tc.tile_pool(name="vaug", bufs=5 if causal else 4) as vaug_pool,
            tc.tile_pool(name="pt", bufs=3 if causal else 2) as pt_pool,
            tc.tile_pool(name="rec", bufs=2) as rec_pool,
            tc.tile_pool(name="osb", bufs=3) as osb_pool,
            tc.tile_pool(name="mrow", bufs=3) as mrow_pool,
            tc.tile_pool(name="st", bufs=2, space="PSUM") as st_pool,
            tc.tile_pool(name="pv", bufs=2, space="PSUM") as pv_pool,
            tc.tile_pool(name="misc", bufs=2, space="PSUM") as misc_pool,
        ):
            ident = const_pool.tile([128, 128], F32)
            make_identity(nc, ident[:])
            tri01 = const_pool.tile([128, 128], BF16)
            make_upper_triangular(nc, tri01[:], val=1.0, diag=True)
            ones = const_pool.tile([1, 64], F32)
            nc.gpsimd.memset(ones[:], 1.0)

            st_w = 1024  # exp chunk width (2 PSUM banks)

            for pair in range(HPC // 2):
                heads = (2 * pair, 2 * pair + 1)

                # ---- Q^T / K^T for the pair: [128, S], rows 0-63 head A's
                # d-dim, rows 64-127 head B's.
                tposed = {}
                for name, src in (("q", q_d), ("k", k_d)):
                    t_tile = (qt_pool if name == "q" else kt_pool).tile(
                        [128, S], F32R)
                    for bank in range(S // 512):
                        ps = misc_pool.tile([128, 512], F32, name="mps", tag="m")
                        stgs = []
                        for j in range(4):
                            qi = 4 * bank + j
                            # both heads side-by-side in the free dim, so one
                            # transpose yields the pair-stacked [d_A; d_B]
                            # layout (transpose outputs must start at
                            # partition 0 on HW)
                            stg = stage_pool.tile([128, 128], F32)
                            for hl in (0, 1):
                                nc.sync.dma_start(
                                    out=stg[:, 64 * hl:64 * (hl + 1)],
                                    in_=src[heads[hl],
                                            128 * qi:128 * (qi + 1), :])
                            stgs.append((j, stg))
                        # the 4 j-transposes share a PSUM bank zero-region;
                        # the start must execute first -> pin the order
                        with tc.tile_critical():
                            for j, stg in stgs:
                                nc.tensor.matmul(
                                    ps[:, 128 * j:128 * (j + 1)],
                                    stg[:], ident[:],
                                    is_transpose=True,
                                    start=(j == 0), stop=(j == 3),
                                )
                        nc.vector.tensor_copy(
                            t_tile[:, 512 * bank:512 * (bank + 1)], ps[:])
                    tposed[name] = t_tile
                qt_t, kt_t = tposed["q"], tposed["k"]

                # ---- V augmented with a ones column, bf16: [128, 65*NT]
                vaug = []
                for hl in (0, 1):
                    va = vaug_pool.tile([128, 65 * NT], BF16)
                    for kt in range(NT):
                        vst = stage_pool.tile([128, 64], F32)
                        nc.sync.dma_start(
                            out=vst[:],
                            in_=v_d[heads[hl], 128 * kt:128 * (kt + 1), :])
                        nc.gpsimd.tensor_copy(
                            va[:, 65 * kt:65 * kt + 64], vst[:])
                        nc.gpsimd.memset(va[:, 65 * kt + 64:65 * kt + 65], 1.0)
                    vaug.append(va)

                # ---- pass 1: S^T chunks -> exp -> P^T (bf16)
                pts = [pt_pool.tile([128, PT_W if causal else NT * S], BF16,
                                    name=f"pt_p{pair}h{hl}", tag="pt")
                       for hl in (0, 1)]
                for kt in range(NT):
                    if causal:
                        w_row = S - 128 * kt  # q in [128*kt, S)
                        q0 = 128 * kt
                        po = _ptoff(kt)
                    else:
                        w_row = S
                        q0 = 0
                        po = kt * S
                    for sub in range(0, w_row, st_w):
                        w = min(st_w, w_row - sub)
                        for hl in (0, 1):
                            stp = st_pool.tile([128, st_w], F32)
                            for o in range(0, w, 512):
                                wm = min(512, w - o)
                                nc.tensor.matmul(
                                    stp[:, o:o + wm],
                                    kt_t[64 * hl:64 * (hl + 1),
                                         128 * kt:128 * (kt + 1)],
                                    qt_t[64 * hl:64 * (hl + 1),
                                         q0 + sub + o:q0 + sub + o + wm],
                                    start=True, stop=True,
                                )
                            if not causal:
                                mrow = mrow_pool.tile([128, st_w], F32)
                                nc.sync.dma_start(
                                    out=mrow[:, 0:w],
                                    in_=mt_d[128 * kt:128 * (kt + 1),
                                             sub:sub + w])
                                nc.vector.tensor_add(
                                    stp[:, 0:w], stp[:, 0:w], mrow[:, 0:w])
                            nc.scalar.activation(
                                pts[hl][:, po + sub:po + sub + w],
                                stp[:, 0:w],
                                mybir.ActivationFunctionType.Exp,
                                scale=SCALE,
                            )
                    if causal:
                        # multiplicative triangular mask on the diagonal block
                        for hl in (0, 1):
                            nc.gpsimd.tensor_mul(
                                pts[hl][:, po:po + 128],
                                pts[hl][:, po:po + 128],
                                tri01[:],
                            )

                # ---- pass 2: PV + normalize, per head / q-block of 512
                for hl in (0, 1):
                    for qb in range(S // 512):
                        kt_hi = (4 * qb + 4) if causal else NT
                        pvp = pv_pool.tile([65, 512], F32)
                        for kt in range(kt_hi):
                            po = _ptoff(kt) if causal else kt * S
                            lo = 512 * qb - (128 * kt if causal else 0)
                            if lo >= 0:
                                rhs = pts[hl][:, po + lo:po + lo + 512]
                                out_ap = pvp[:, 0:512]
                            else:
                                # diagonal-crossing tile: starts mid-block
                                wpart = 512 + lo  # lo negative
                                rhs = pts[hl][:, po:po + wpart]
                                out_ap = pvp[:, -lo:512]
                            nc.tensor.matmul(
                                out_ap,
                                vaug[hl][:, 65 * kt:65 * kt + 65],
                                rhs,
                                start=(kt == 0), stop=(kt == kt_hi - 1),
                            )
                        rec = rec_pool.tile([1, 512], F32)
                        nc.vector.reciprocal(rec[:], pvp[64:65, :])
                        rrep = rec_pool.tile([64, 512], F32, name="rrep",
                                             tag="rrep")
                        nc.gpsimd.partition_broadcast(rrep[:], rec[:])
                        ot = osb_pool.tile([64, 512], F32)
                        nc.vector.tensor_mul(ot[:], pvp[0:64, :], rrep[:])
                        nc.sync.dma_start(
                            out=out_d[heads[hl], :, 512 * qb:512 * (qb + 1)],
                            in_=ot[:])

    nc.compile()
    return nc


_CACHE: dict = {}


def _get_nc(causal: bool) -> bacc.Bacc:
    if causal not in _CACHE:
        _CACHE[causal] = _build(causal)
    return _CACHE[causal]


def _is_canonical_causal(mask: np.ndarray) -> bool:
    if mask.shape != (B, 1, S, S):
        return False
    tri = np.triu(np.ones((S, S), dtype=bool), k=1)
    m0 = mask[0, 0]
    if not (np.all(m0[~tri] == 0.0) and np.all(m0[tri] <= -1e8)):
        return False
    return bool(np.array_equal(mask[0, 0], mask[1, 0]))


def kernel(query_states, key_states, value_states, causal_attention_mask):
    q = np.ascontiguousarray(np.asarray(query_states, dtype=np.float32))
    k = np.ascontiguousarray(np.asarray(key_states, dtype=np.float32))
    v = np.ascontiguousarray(np.asarray(value_states, dtype=np.float32))
    mask = np.asarray(causal_attention_mask, dtype=np.float32)

    causal = _is_canonical_causal(mask)
    nc = _get_nc(causal)

    def heads_of(x):
        # [B, S, H*D] -> [B*H, S, D]
        return np.ascontiguousarray(
            x.reshape(B, S, H, D).transpose(0, 2, 1, 3).reshape(B * H, S, D))

    qh, kh, vh = heads_of(q), heads_of(k), heads_of(v)
    in_maps = []
    for c in range(N_CORES):
        m = {
            "q": qh[HPC * c:HPC * (c + 1)],
            "k": kh[HPC * c:HPC * (c + 1)],
            "v": vh[HPC * c:HPC * (c + 1)],
        }
        if not causal:
            b = (HPC * c) // H
            # pre-scale by 1/SCALE: device computes exp((S + maskT)*SCALE)
            m["maskT"] = np.ascontiguousarray(mask[b, 0].T) / SCALE
        in_maps.append(m)

    res = run_bass_kernel_spmd(nc, in_maps, list(range(N_CORES)))

    out = np.empty((B * H, S, D), dtype=np.float32)
    for c in range(N_CORES):
        ot = res.results[c]["outT"]  # [HPC, D, S]
        for hl in range(HPC):
            out[HPC * c + hl] = ot[hl].T
    # [B*H, S, D] -> [B, S, H*D]
    return np.ascontiguousarray(
        out.reshape(B, H, S, D).transpose(0, 2, 1, 3).reshape(B, S, EMBED))


# revision 39
# speedup vs baseline: 5.7529x; 5.7529x over previous
"""Trainium2 Bass kernel for multi-head causal attention.

Problem: B=2, H=16, S=2048, D=64, fp32, additive causal mask.
Sharding: B*H = 32 heads -> 4 heads per core across 8 cores (no cross-core
communication).

Per-core algorithm (heads processed in row-tile-packed pairs):
  - Q^T, K^T [64, S] built on-chip via PE transposes (fp32 -> fp32r rounded
    by the DVE PSUM->SBUF copy).
  - Scores are computed TRANSPOSED: S^T[k, q] = (K^T)^T-chunk.T @ Q^T via
    fp32r matmuls, two heads packed in the 128-row PE array (contraction
    dim is d=64 per head).  Causally trimmed: for k-tile kt only
    q >= 128*kt is computed.
  - exp via ACT (scale=1/8 folded in, no max-subtraction needed: scores are
    O(6)), writing P^T tiles in bf16.  Diagonal 128x128 blocks get a
    multiplicative upper-triangular 0/1 mask.
  - PV: out^T[d, q] accumulated in PSUM over k-chunks with V (as stored,
    bf16, augmented with a ones column -> row 64 is softmax denominator).
  - normalize: reciprocal of row 64, replicated across partitions with a
    K=1 ones matmul, multiplied on DVE; host transposes [d, q] -> [q, d].
"""

import numpy as np

import concourse.bass as bass
import concourse.mybir as mybir
import concourse.tile as tile
from concourse import bacc
from concourse.bass_utils import run_bass_kernel_spmd
from concourse.masks import make_identity, make_upper_triangular

B = 2
H = 16
S = 2048
D = 64
EMBED = H * D
N_CORES = 8
HPC = (B * H) // N_CORES  # heads per core = 4
NT = S // 128  # 16 k/q tiles of 128
SCALE = float(D) ** -0.5  # 0.125
NEG = -1e9

F32 = mybir.dt.float32
F32R = mybir.dt.float32r
BF16 = mybir.dt.bfloat16


def _ptoff(kt: int) -> int:
    """Column offset of k-tile kt's row-chunk inside a per-head P^T tile.

    Chunk kt covers global q in [128*kt, S) and is stored at local offset
    q - 128*kt."""
    return kt * S - 128 * (kt * (kt - 1) // 2)


PT_W = _ptoff(NT)  # 17408 columns total (causal)


def _build(causal: bool = True, reps: int = 1) -> bacc.Bacc:
    nc = bacc.Bacc("TRN2", target_bir_lowering=False, debug=False,
                   num_devices=N_CORES)

    q_d = nc.declare_dram_parameter("q", [HPC, S, D], F32, isOutput=False)
    k_d = nc.declare_dram_parameter("k", [HPC, S, D], F32, isOutput=False)
    v_d = nc.declare_dram_parameter("v", [HPC, S, D], F32, isOutput=False)
    if not causal:
        # mask^T for this core's batch: maskT[k, q] = mask[b, 0, q, k]
        mt_d = nc.declare_dram_parameter("maskT", [S, S], F32, isOutput=False)
    out_d = nc.declare_dram_parameter("outT", [HPC, D, S], F32, isOutput=True)

    with tile.TileContext(nc) as tc:
        with (
            tc.tile_pool(name="const", bufs=1) as const_pool,
            tc.tile_pool(name="stage", bufs=4) as stage_pool,
            tc.tile_pool(name="qt", bufs=2 if causal else 1) as qt_pool,
            tc.tile_pool(name="ktp", bufs=2 if causal else 1) as kt_pool,
            tc.tile_pool(name="vaug", bufs=5 if causal else 4) as vaug_pool,
            tc.tile_pool(name="pt", bufs=3 if causal else 2) as pt_pool,
            tc.tile_pool(name="rec", bufs=2) as rec_pool,
            tc.tile_pool(name="osb", bufs=3) as osb_pool,
            tc.tile_pool(name="mrow", bufs=3) as mrow_pool,
            tc.tile_pool(name="st", bufs=3, space="PSUM") as st_pool,
            tc.tile_pool(name="aux", bufs=2, space="PSUM") as aux_pool,
        ):
            ident = const_pool.tile([128, 128], F32)
            make_identity(nc, ident[:])
            tri01 = const_pool.tile([128, 128], BF16)
            make_upper_triangular(nc, tri01[:], val=1.0, diag=True)
            ones = const_pool.tile([1, 64], F32)
            nc.gpsimd.memset(ones[:], 1.0)

            st_w = 1024  # exp chunk width (2 PSUM banks)

            n_rp = reps * (HPC // 2)
            built = {}

            def emit_build(rp):
                """Q^T / K^T for pair rp%2: [128, S], rows 0-63 head A's
                d-dim, rows 64-127 head B's.  Dependency-ordered: kt=0's
                highest sub-chunk needs K bank 0 and Q banks 3,2 first;
                later k-tiles need K banks in order."""
                p = rp % (HPC // 2)
                hds = (2 * p, 2 * p + 1)
                qt_t = qt_pool.tile([128, S], F32R, name=f"qt{rp}", tag="qt")
                kt_t = kt_pool.tile([128, S], F32R, name=f"kt{rp}", tag="kt")
                build_order = [(kt_t, k_d, 0), (qt_t, q_d, 3), (qt_t, q_d, 2),
                               (qt_t, q_d, 1), (qt_t, q_d, 0), (kt_t, k_d, 1),
                               (kt_t, k_d, 2), (kt_t, k_d, 3)]
                for t_tile, src, bank in build_order:
                    ps = aux_pool.tile([128, 512], F32, name="mps", tag="m")
                    # one batched DMA stages 4 q-tiles x (2 heads x d):
                    # stg[:, 128j + 64hl + d] = src[head hl, q, d]; each
                    # transpose then yields the pair-stacked [d_A; d_B]
                    # layout (transpose outputs must start at partition 0)
                    stg = stage_pool.tile([128, 512], F32)
                    for hl in (0, 1):
                        nc.sync.dma_start(
                            out=stg[:]
                            .rearrange("p (j h d) -> p j h d", j=4, h=2)
                            [:, :, hl, :],
                            in_=src[hds[hl],
                                    512 * bank:512 * (bank + 1), :]
                            .rearrange("(j p) d -> p j d", p=128),
                        )
                    # the 4 j-transposes share a PSUM bank zero-region;
                    # the start must execute first -> pin the order
                    with tc.tile_critical():
                        for j in range(4):
                            nc.tensor.matmul(
                                ps[:, 128 * j:128 * (j + 1)],
                                stg[:, 128 * j:128 * (j + 1)],
                                ident[:],
                                is_transpose=True,
                                start=(j == 0), stop=(j == 3),
                            )
                    nc.vector.tensor_copy(
                        t_tile[:, 512 * bank:512 * (bank + 1)], ps[:])
                built[rp] = (qt_t, kt_t)

            for rep_pair in range(n_rp):
                pair = rep_pair % (HPC // 2)
                heads = (2 * pair, 2 * pair + 1)
                if rep_pair == 0:
                    emit_build(0)
                qt_t, kt_t = built.pop(rep_pair)

                # ---- V augmented with a ones column, bf16: [128, 65*NT].
                # Emitted late (at kt==1) so the V DMAs don't compete with
                # the Q/K builds feeding the first exps.
                vaug = []

                def emit_v_build():
                    for hl in (0, 1):
                        vst = stage_pool.tile([128, NT * 64], F32, name="vst",
                                              tag="vst", bufs=2)
                        nc.sync.dma_start(
                            out=vst[:].rearrange("p (n d) -> p n d", n=NT),
                            in_=v_d[heads[hl]].rearrange("(n p) d -> p n d",
                                                         p=128),
                        )
                        va = vaug_pool.tile([128, 65 * NT], BF16, name="va",
                                            tag="va")
                        # strided cast f32 -> bf16, leaving ones-column gaps
                        nc.gpsimd.tensor_copy(
                            va[:].rearrange("p (n e) -> p n e", e=65)
                            [:, :, 0:64],
                            vst[:].rearrange("p (n d) -> p n d", n=NT),
                        )
                        nc.gpsimd.memset(
                            va[:].rearrange("p (n e) -> p n e", e=65)
                            [:, :, 64:65],
                            1.0)
                        vaug.append(va)

                # ---- pass 1: S^T chunks -> exp -> P^T (bf16)
                pts = [pt_pool.tile([128, PT_W if causal else NT * S], BF16,
                                    name=f"pt_p{pair}h{hl}", tag="pt")
                       for hl in (0, 1)]

                def emit_pv(hl, qb, kt_hi):
                    """PV accumulation + softmax normalization for one
                    512-wide q-block (requires PT k-tiles < kt_hi)."""
                    pvp = aux_pool.tile([128, 512], F32, name="pvp", tag="m")[0:65, :]
                    for kt in range(kt_hi):
                        po = _ptoff(kt) if causal else kt * S
                        lo = 512 * qb - (128 * kt if causal else 0)
                        if lo >= 0:
                            rhs = pts[hl][:, po + lo:po + lo + 512]
                            out_ap = pvp[:, 0:512]
                        else:
                            # diagonal-crossing tile: starts mid-block
                            wpart = 512 + lo  # lo negative
                            rhs = pts[hl][:, po:po + wpart]
                            out_ap = pvp[:, -lo:512]
                        nc.tensor.matmul(
                            out_ap,
                            vaug[hl][:, 65 * kt:65 * kt + 65],
                            rhs,
                            start=(kt == 0), stop=(kt == kt_hi - 1),
                        )
                    rec = rec_pool.tile([1, 512], F32, name="rec", tag="rec")
                    nc.vector.reciprocal(rec[:], pvp[64:65, :])
                    rrep = rec_pool.tile([64, 512], F32, name="rrep",
                                         tag="rrep")
                    nc.gpsimd.partition_broadcast(rrep[:], rec[:])
                    ot = osb_pool.tile([64, 512], F32, name="ot", tag="ot")
                    nc.vector.tensor_mul(ot[:], pvp[0:64, :], rrep[:])
                    nc.sync.dma_start(
                        out=out_d[heads[hl], :, 512 * qb:512 * (qb + 1)],
                        in_=ot[:])

                kt_seq = list(range(NT))
                for kt in kt_seq:
                    if kt == 1 or (kt == 0 and not causal):
                        emit_v_build()
                    # prefetch the next pair's Q^T/K^T build ahead of this
                    # pair's last PV chains in priority order
                    if kt == 12 and rep_pair + 1 < n_rp:
                        emit_build(rep_pair + 1)
                    if causal:
                        w_row = S - 128 * kt  # q in [128*kt, S)
                        q0 = 128 * kt
                        po = _ptoff(kt)
                    else:
                        w_row = S
                        q0 = 0
                        po = kt * S
                    # descending sub order: the high-q chunk's Q banks are
                    # built first
                    for sub in reversed(range(0, w_row, st_w)):
                        w = min(st_w, w_row - sub)
                        for hl in (0, 1):
                            stp = st_pool.tile([128, st_w], F32)
                            for o in range(0, w, 512):
                                wm = min(512, w - o)
                                nc.tensor.matmul(
                                    stp[:, o:o + wm],
                                    kt_t[64 * hl:64 * (hl + 1),
                                         128 * kt:128 * (kt + 1)],
                                    qt_t[64 * hl:64 * (hl + 1),
                                         q0 + sub + o:q0 + sub + o + wm],
                                    start=True, stop=True,
                                )
                            if not causal:
                                mrow = mrow_pool.tile([128, st_w], F32)
                                nc.sync.dma_start(
                                    out=mrow[:, 0:w],
                                    in_=mt_d[128 * kt:128 * (kt + 1),
                                             sub:sub + w])
                                nc.vector.tensor_add(
                                    stp[:, 0:w], stp[:, 0:w], mrow[:, 0:w])
                            nc.scalar.activation(
                                pts[hl][:, po + sub:po + sub + w],
                                stp[:, 0:w],
                                mybir.ActivationFunctionType.Exp,
                                scale=SCALE,
                            )
                    if causal:
                        # multiplicative triangular mask on the diagonal block
                        for hl in (0, 1):
                            nc.gpsimd.tensor_mul(
                                pts[hl][:, po:po + 128],
                                pts[hl][:, po:po + 128],
                                tri01[:],
                            )
                        # q-block kt//4 has all its k-tiles -> fire PV now,
                        # keeping PE/DVE busy while ACT keeps exp-ing
                        if kt % 4 == 3:
                            for hl in (0, 1):
                                emit_pv(hl, kt // 4, kt + 1)

                if not causal:
                    for hl in (0, 1):
                        for qb in range(S // 512):
                            emit_pv(hl, qb, NT)

    nc.compile()
    return nc


_CACHE: dict = {}


def _get_nc(causal: bool) -> bacc.Bacc:
    if causal not in _CACHE:
        _CACHE[causal] = _build(causal)
    return _CACHE[causal]


def _is_canonical_causal(mask: np.ndarray) -> bool:
    if mask.shape != (B, 1, S, S):
        return False
    tri = np.triu(np.ones((S, S), dtype=bool), k=1)
    m0 = mask[0, 0]
    if not (np.all(m0[~tri] == 0.0) and np.all(m0[tri] <= -1e8)):
        return False
    return bool(np.array_equal(mask[0, 0], mask[1, 0]))


def kernel(query_states, key_states, value_states, causal_attention_mask):
    q = np.ascontiguousarray(np.asarray(query_states, dtype=np.float32))
    k = np.ascontiguousarray(np.asarray(key_states, dtype=np.float32))
    v = np.ascontiguousarray(np.asarray(value_states, dtype=np.float32))
    mask = np.asarray(causal_attention_mask, dtype=np.float32)

    causal = _is_canonical_causal(mask)
    nc = _get_nc(causal)

    def heads_of(x):
        # [B, S, H*D] -> [B*H, S, D]
        return np.ascontiguousarray(
            x.reshape(B, S, H, D).transpose(0, 2, 1, 3).reshape(B * H, S, D))

    qh, kh, vh = heads_of(q), heads_of(k), heads_of(v)
    in_maps = []
    for c in range(N_CORES):
        m = {
            "q": qh[HPC * c:HPC * (c + 1)],
            "k": kh[HPC * c:HPC * (c + 1)],
            "v": vh[HPC * c:HPC * (c + 1)],
        }
        if not causal:
            b = (HPC * c) // H
            # pre-scale by 1/SCALE: device computes exp((S + maskT)*SCALE)
            m["maskT"] = np.ascontiguousarray(mask[b, 0].T) / SCALE
        in_maps.append(m)

    res = run_bass_kernel_spmd(nc, in_maps, list(range(N_CORES)))

    out = np.empty((B * H, S, D), dtype=np.float32)
    for c in range(N_CORES):
        ot = res.results[c]["outT"]  # [HPC, D, S]
        for hl in range(HPC):
            out[HPC * c + hl] = ot[hl].T
    # [B*H, S, D] -> [B, S, H*D]
    return np.ascontiguousarray(
        out.reshape(B, H, S, D).transpose(0, 2, 1, 3).reshape(B, S, EMBED))
